# revision 3
# baseline (speedup 1.0000x reference)
"""L1 loss (mean |yhat - y|) over (64, 128, 4096) fp32 tensors on 8 TRN2 cores.

v4: data-parallel batch shard; flat per-core [128, 32768] view (sum is
permutation-invariant so the zero-copy flat re-chunk is valid). yhat loads
on the Sync HWDGE ring, y on the Scalar ring. Tile widths taper
(7x4096, 2048, 1024, 512, 256, 256) so the post-stream compute tail is
tiny. Tapered tiles get dedicated SBUF slots (bufs=1 pools, unique tags)
so their DMAs enqueue with no dependency on big-tile compute - the DMA
rings never go idle waiting on slot releases. Big tiles: DVE sub +
abs-sum-reduce. Small tiles: DVE sub + ScalarE activation(Abs,
accum_out) so the DVE is not the pacer during the final burst of small
arrivals. Host sums partials in float64.
"""

import numpy as np

import concourse.bacc as bacc
import concourse.bass as bass
import concourse.mybir as mybir
import concourse.tile as tile
from concourse.bass_utils import run_bass_kernel_spmd

N_CORES = 8
FULL_SHAPE = (64, 128, 4096)
TOTAL_ELEMS = FULL_SHAPE[0] * FULL_SHAPE[1] * FULL_SHAPE[2]  # 33,554,432

P = 128
ELEMS_PER_CORE = TOTAL_ELEMS // N_CORES   # 4,194,304
F_TOTAL = ELEMS_PER_CORE // P             # 32,768

F_BIG = [4096] * 7
F_SMALL = [2048, 1024, 512, 256, 256]
F_TILES = F_BIG + F_SMALL
assert sum(F_TILES) == F_TOTAL
N_TILES = len(F_TILES)

_nc_cache = []


def _build_nc():
    nc = bacc.Bacc("TRN2", target_bir_lowering=False, debug=False)
    yh = nc.declare_dram_parameter("yh", [P, F_TOTAL], mybir.dt.float32, isOutput=False)
    yy = nc.declare_dram_parameter("yy", [P, F_TOTAL], mybir.dt.float32, isOutput=False)
    out = nc.declare_dram_parameter("out", [P, N_TILES], mybir.dt.float32, isOutput=True)

    with tile.TileContext(nc) as tc:
        with (
            tc.tile_pool(name="ina", bufs=3) as a_pool,
            tc.tile_pool(name="inb", bufs=3) as b_pool,
            tc.tile_pool(name="diff", bufs=2) as diff_pool,
            tc.tile_pool(name="small", bufs=1) as small_pool,
            tc.tile_pool(name="acc", bufs=1) as acc_pool,
        ):
            acc = acc_pool.tile([P, N_TILES], mybir.dt.float32)
            sc = small_pool.tile([P, F_SMALL[0]], mybir.dt.float32, tag="scratch")
            off = 0
            for i, f in enumerate(F_TILES):
                big = f == 4096
                if big:
                    at = a_pool.tile([P, f], mybir.dt.float32, tag="a")
                    bt = b_pool.tile([P, f], mybir.dt.float32, tag="b")
                    d = diff_pool.tile([P, f], mybir.dt.float32, tag="d")
                else:
                    at = small_pool.tile([P, f], mybir.dt.float32, tag=f"a{i}")
                    bt = small_pool.tile([P, f], mybir.dt.float32, tag=f"b{i}")
                    d = small_pool.tile([P, f], mybir.dt.float32, tag=f"d{i}")
                nc.sync.dma_start(at[:], yh[:, off : off + f])
                nc.scalar.dma_start(bt[:], yy[:, off : off + f])
                nc.vector.tensor_sub(d[:], at[:], bt[:])
                if big:
                    nc.vector.tensor_reduce(
                        acc[:, i : i + 1],
                        d[:],
                        axis=mybir.AxisListType.X,
                        op=mybir.AluOpType.add,
                        apply_absolute_value=True,
                    )
                else:
                    nc.scalar.activation(
                        sc[:, 0:f],
                        d[:],
                        mybir.ActivationFunctionType.Abs,
                        accum_out=acc[:, i : i + 1],
                    )
                off += f
            nc.sync.dma_start(out[:], acc[:])
    nc.compile()
    return nc


def _get_nc():
    if not _nc_cache:
        _nc_cache.append(_build_nc())
    return _nc_cache[0]


def _shard_inputs(yhat: np.ndarray, y: np.ndarray) -> list[dict[str, np.ndarray]]:
    yh = np.ascontiguousarray(yhat, dtype=np.float32).reshape(N_CORES, P, F_TOTAL)
    yy = np.ascontiguousarray(y, dtype=np.float32).reshape(N_CORES, P, F_TOTAL)
    return [{"yh": yh[c], "yy": yy[c]} for c in range(N_CORES)]


def kernel(yhat: np.ndarray, y: np.ndarray) -> np.ndarray:
    nc = _get_nc()
    in_maps = _shard_inputs(yhat, y)
    res = run_bass_kernel_spmd(nc, in_maps, list(range(N_CORES)))
    total = np.float64(0.0)
    for r in res.results:
        total += r["out"].astype(np.float64).sum()
    return np.asarray(total / TOTAL_ELEMS, dtype=np.float32)


# revision 6
# speedup vs baseline: 1.1173x; 1.1173x over previous
"""L1 loss (mean |yhat - y|) over (64, 128, 4096) fp32 tensors on 8 TRN2 cores.

v6: data-parallel batch shard; flat per-core [128, 32768] view (the global
sum is permutation-invariant so the zero-copy flat re-chunk is valid).
Tile widths taper (7x4096, 2048, 1024, 512, 256, 256) and the tapered
tiles get dedicated SBUF slots so their DMAs enqueue without waiting on
compute.

All input loads go on the Sync HWDGE ring, interleaved a0,b0,a1,b1,...:
the SP engine issues only DMAs so no compute ever stalls an issue, and
both halves of a tile arrive adjacently (no end-of-stream ring imbalance).
Engine split keeps every engine well under the DMA arrival rate
(~10-12us per 4-MiB tile pair): the vector engine does ONLY the subtract
(~4.4us/tile); the scalar engine does abs + per-partition sum in a single
activation(Abs, accum_out) pass (~3.6us/tile, HW-validated exact). The
accumulator is written only by ScalarE, so the final out-DMA (on the
scalar ring) needs no cross-engine sync. Host sums the partials in
float64.
"""

import numpy as np

import concourse.bacc as bacc
import concourse.bass as bass
import concourse.mybir as mybir
import concourse.tile as tile
from concourse.bass_utils import run_bass_kernel_spmd

N_CORES = 8
FULL_SHAPE = (64, 128, 4096)
TOTAL_ELEMS = FULL_SHAPE[0] * FULL_SHAPE[1] * FULL_SHAPE[2]  # 33,554,432

P = 128
ELEMS_PER_CORE = TOTAL_ELEMS // N_CORES   # 4,194,304
F_TOTAL = ELEMS_PER_CORE // P             # 32,768

F_BIG = [4096] * 7
F_SMALL = [2048, 1024, 512, 256, 256]
F_TILES = F_BIG + F_SMALL
assert sum(F_TILES) == F_TOTAL
N_TILES = len(F_TILES)

_nc_cache = []


def _build_nc():
    nc = bacc.Bacc("TRN2", target_bir_lowering=False, debug=False)
    yh = nc.declare_dram_parameter("yh", [P, F_TOTAL], mybir.dt.float32, isOutput=False)
    yy = nc.declare_dram_parameter("yy", [P, F_TOTAL], mybir.dt.float32, isOutput=False)
    out = nc.declare_dram_parameter("out", [P, N_TILES], mybir.dt.float32, isOutput=True)

    with tile.TileContext(nc) as tc:
        with (
            tc.tile_pool(name="ina", bufs=3) as a_pool,
            tc.tile_pool(name="inb", bufs=3) as b_pool,
            tc.tile_pool(name="diff", bufs=2) as diff_pool,
            tc.tile_pool(name="small", bufs=1) as small_pool,
            tc.tile_pool(name="acc", bufs=1) as acc_pool,
        ):
            acc = acc_pool.tile([P, N_TILES], mybir.dt.float32)
            sc = acc_pool.tile([P, 4096], mybir.dt.float32, tag="scratch")
            off = 0
            for i, f in enumerate(F_TILES):
                big = f == 4096
                if big:
                    at = a_pool.tile([P, f], mybir.dt.float32, tag="a")
                    bt = b_pool.tile([P, f], mybir.dt.float32, tag="b")
                else:
                    at = small_pool.tile([P, f], mybir.dt.float32, tag=f"a{i}")
                    bt = small_pool.tile([P, f], mybir.dt.float32, tag=f"b{i}")
                d = diff_pool.tile([P, f], mybir.dt.float32, tag="d")
                nc.sync.dma_start(at[:], yh[:, off : off + f])
                nc.sync.dma_start(bt[:], yy[:, off : off + f])
                nc.vector.tensor_sub(d[:], at[:], bt[:])
                nc.scalar.activation(
                    sc[:, 0:f],
                    d[:],
                    mybir.ActivationFunctionType.Abs,
                    accum_out=acc[:, i : i + 1],
                )
                off += f
            nc.scalar.dma_start(out[:], acc[:])
    nc.compile()
    return nc


def _get_nc():
    if not _nc_cache:
        _nc_cache.append(_build_nc())
    return _nc_cache[0]


def _shard_inputs(yhat: np.ndarray, y: np.ndarray) -> list[dict[str, np.ndarray]]:
    yh = np.ascontiguousarray(yhat, dtype=np.float32).reshape(N_CORES, P, F_TOTAL)
    yy = np.ascontiguousarray(y, dtype=np.float32).reshape(N_CORES, P, F_TOTAL)
    return [{"yh": yh[c], "yy": yy[c]} for c in range(N_CORES)]


def kernel(yhat: np.ndarray, y: np.ndarray) -> np.ndarray:
    nc = _get_nc()
    in_maps = _shard_inputs(yhat, y)
    res = run_bass_kernel_spmd(nc, in_maps, list(range(N_CORES)))
    total = np.float64(0.0)
    for r in res.results:
        total += r["out"].astype(np.float64).sum()
    return np.asarray(total / TOTAL_ELEMS, dtype=np.float32)
